# revision 11
# baseline (speedup 1.0000x reference)
"""Trainium2 Bass kernel for nn_Attention_17995912970332.

Full attention layer: QKV proj + per-head RMSNorm(q,k) + RoPE + softmax
attention + output projection. B=2, N=2048, C=1024, H=16 heads, D=64.

Sharding: 8 cores = 2 (batch) x 4 (head groups of 4 heads). Each core
computes its 4 heads' attention and a partial output projection
(row-parallel Wo); host sums the 4 partials per batch and adds bias.

Structure (software-pipelined):
  Pass A: K projection for all 16 token tiles -> rmsnorm+rope -> PE-transpose
          into kt [d, pair, ntok].  (x^T streamed from HBM)
  Pass B: Q + V projections per token tile (x^T streamed again); qt filled
          progressively so attention stripe s=0 starts once tiles 0-7 done.
  Attention (stripe-outer): for s in {0,1}: for h in {0..3}:
      S^T = kt_i^T-contract matmul (K=64, fp32r), exp on ScalarE
      (scale=1/8 folded) -> bf16 E; PV with lhsT=[V|1] bf16 -> po rows 0..63 =
      unnormalized O^T, row 64 = softmax denom l; linv=reciprocal_approx_fast;
      broadcast linv across 64 partitions with a partition-stride-0 SBUF DMA;
      xat = po * linv.
    then output projection for the 8 token tiles of stripe s (fp32r) -> DMA.
"""

import sys

import numpy as np

sys.path.insert(0, "/opt/trn_rl_repo")

H, D, EPS = 16, 64, 1e-6
SCALE = D**-0.5
B, N, C = 2, 2048, 1024
HG = 4          # heads per core
GW = HG * D     # 256, group width
NT = N // 128   # 16 token tiles
KA = C // 128   # 8 contraction tiles for qkv proj

_CACHE = {}


def _build():
    import concourse.bass as bass
    import concourse.tile as tile
    from concourse import bacc, mybir
    from concourse.masks import make_identity

    f32 = mybir.dt.float32
    f32r = mybir.dt.float32r
    bf16 = mybir.dt.bfloat16
    mult = mybir.AluOpType.mult
    add = mybir.AluOpType.add

    nc = bacc.Bacc("TRN2", target_bir_lowering=False, debug=False)

    xt_d = nc.dram_tensor("xt", [NT, KA, 128, 128], f32r, kind="ExternalInput").ap()
    wq_d = nc.dram_tensor("wq", [KA, 128, GW], f32r, kind="ExternalInput").ap()
    wk_d = nc.dram_tensor("wk", [KA, 128, GW], f32r, kind="ExternalInput").ap()
    wv_d = nc.dram_tensor("wv", [KA, 128, GW], f32r, kind="ExternalInput").ap()
    wo_d = nc.dram_tensor("wo", [2, 128, C], f32r, kind="ExternalInput").ap()
    cosq_d = nc.dram_tensor("cosq", [NT, 128, D], f32, kind="ExternalInput").ap()
    sinq_d = nc.dram_tensor("sinq", [NT, 128, D], f32, kind="ExternalInput").ap()
    cosk_d = nc.dram_tensor("cosk", [NT, 128, D], f32, kind="ExternalInput").ap()
    sink_d = nc.dram_tensor("sink", [NT, 128, D], f32, kind="ExternalInput").ap()
    out_d = nc.dram_tensor("out", [N, C], f32, kind="ExternalOutput").ap()

    def bc_h(ap2d, reps=HG):
        # [128, X] AP -> [128, reps, X] with stride-0 middle dim (broadcast)
        return bass.AP(
            tensor=ap2d.tensor,
            offset=ap2d.offset,
            ap=[list(ap2d.ap[0]), [0, reps], list(ap2d.ap[1])],
        )

    with tile.TileContext(nc) as tc:
        with (
            tc.tile_pool(name="consts", bufs=1) as consts,
            tc.tile_pool(name="weights", bufs=1) as wpool,
            tc.tile_pool(name="qkt", bufs=1) as qkt,
            tc.tile_pool(name="vbuf", bufs=1) as vbuf,
            tc.tile_pool(name="xat", bufs=1) as xatp,
            tc.tile_pool(name="cs4", bufs=1) as cs4,
        ):
            ident = consts.tile([128, 128], f32)
            make_identity(nc, ident)
            eps128 = consts.tile([128, 1], f32)
            nc.vector.memset(eps128, EPS)

            wq_sb = wpool.tile([128, KA, GW], f32r, tag="wq")
            wk_sb = wpool.tile([128, KA, GW], f32r, tag="wk")
            wv_sb = wpool.tile([128, KA, GW], f32r, tag="wv")
            wo_sb = wpool.tile([128, 2, C], f32r, tag="wo")
            nc.sync.dma_start(out=wk_sb, in_=wk_d.rearrange("a p n -> p a n"))
            nc.sync.dma_start(out=wq_sb, in_=wq_d.rearrange("a p n -> p a n"))
            nc.sync.dma_start(out=wv_sb, in_=wv_d.rearrange("a p n -> p a n"))
            nc.sync.dma_start(out=wo_sb, in_=wo_d.rearrange("a p n -> p a n"))

            cosq_sb = cs4.tile([128, NT, D], f32, tag="cq")
            sinq_sb = cs4.tile([128, NT, D], f32, tag="sq")
            cosk_sb = cs4.tile([128, NT, D], f32, tag="ck")
            sink_sb = cs4.tile([128, NT, D], f32, tag="sk")
            nc.sync.dma_start(out=cosk_sb, in_=cosk_d.rearrange("t p d -> p t d"))
            nc.sync.dma_start(out=sink_sb, in_=sink_d.rearrange("t p d -> p t d"))
            nc.sync.dma_start(out=cosq_sb, in_=cosq_d.rearrange("t p d -> p t d"))
            nc.sync.dma_start(out=sinq_sb, in_=sinq_d.rearrange("t p d -> p t d"))

            qt_sb = qkt.tile([128, 2, N], f32, tag="qt")   # q^T: p=(h%2)*64+d, j=h//2
            kt_sb = qkt.tile([128, 2, N], f32, tag="kt")
            xat_sb = xatp.tile([128, 2, N], f32, tag="xat")
            vp_sb = vbuf.tile([128, NT, HG, 66], bf16, tag="vp")
            nc.vector.memset(vp_sb, 1.0)

            # -------- Phase 1: two passes over x (K first, then Q+V) --------
            with (
                tc.tile_pool(name="xtok", bufs=3) as xpool,
                tc.tile_pool(name="p1w", bufs=3) as p1w,
                tc.tile_pool(name="p1s", bufs=4) as p1s,
                tc.tile_pool(name="ps_qkv", bufs=2, space="PSUM") as pqkv,
                tc.tile_pool(name="ps_tr", bufs=2, space="PSUM") as ptr,
            ):

                def norm_rope_transpose(p, nm, cos_sb, sin_sb, dst, t):
                    p3 = p.rearrange("p (h d) -> p h d", h=HG)
                    sq = p1s.tile([128, GW], f32, tag="sq")
                    nc.scalar.activation(
                        out=sq, in_=p, func=mybir.ActivationFunctionType.Square
                    )
                    ms = p1s.tile([128, HG], f32, tag="ms")
                    nc.vector.tensor_reduce(
                        out=ms,
                        in_=sq.rearrange("p (h d) -> p h d", h=HG),
                        axis=mybir.AxisListType.X,
                        op=add,
                    )
                    rstd = p1s.tile([128, HG], f32, tag="rstd")
                    nc.scalar.activation(
                        out=rstd,
                        in_=ms,
                        func=mybir.ActivationFunctionType.Sqrt,
                        scale=1.0 / D,
                        bias=eps128,
                    )
                    nc.vector.reciprocal(out=rstd, in_=rstd)
                    rotm = p1w.tile([128, HG, D], f32, tag="rotm" + nm)
                    nc.vector.tensor_tensor(
                        out=rotm[:, :, 0:32],
                        in0=p3[:, :, 32:64],
                        in1=bc_h(sin_sb[:, t, 0:32]),
                        op=mult,
                    )
                    nc.vector.tensor_tensor(
                        out=rotm[:, :, 32:64],
                        in0=p3[:, :, 0:32],
                        in1=bc_h(sin_sb[:, t, 32:64]),
                        op=mult,
                    )
                    qc = p1w.tile([128, HG, D], f32, tag="qc" + nm)
                    nc.vector.tensor_tensor(
                        out=qc, in0=p3, in1=bc_h(cos_sb[:, t, :]), op=mult
                    )
                    qf = p1w.tile([128, HG, D], f32, tag="qf" + nm)
                    nc.gpsimd.tensor_tensor(out=qf, in0=qc, in1=rotm, op=add)
                    for h in range(HG):
                        nc.vector.tensor_scalar_mul(
                            qf[:, h, :], qf[:, h, :], rstd[:, h : h + 1]
                        )
                    qf2 = qf.rearrange("p h d -> p (h d)")
                    for half in range(2):
                        ptile = ptr.tile([128, 128], f32, tag="tr")
                        nc.tensor.transpose(
                            out=ptile,
                            in_=qf2[:, half * 128 : half * 128 + 128],
                            identity=ident,
                        )
                        nc.vector.tensor_copy(
                            out=dst[:, half, t * 128 : t * 128 + 128].bitcast(f32r),
                            in_=ptile,
                        )

                # Pass A: K for all tiles
                for t in range(NT):
                    xtok = xpool.tile([128, KA, 128], f32r, tag="xtok")
                    nc.sync.dma_start(
                        out=xtok, in_=xt_d[t].rearrange("a p n -> p a n")
                    )
                    p = pqkv.tile([128, GW], f32, tag="k")
                    for a in range(KA):
                        nc.tensor.matmul(
                            p,
                            lhsT=xtok[:, a, :],
                            rhs=wk_sb[:, a, :],
                            start=(a == 0),
                            stop=(a == KA - 1),
                        )
                    norm_rope_transpose(p, "k", cosk_sb, sink_sb, kt_sb, t)

                # Pass B: Q + V per tile
                for t in range(NT):
                    xtok = xpool.tile([128, KA, 128], f32r, tag="xtok")
                    nc.sync.dma_start(
                        out=xtok, in_=xt_d[t].rearrange("a p n -> p a n")
                    )
                    pq = pqkv.tile([128, GW], f32, tag="q")
                    pv = pqkv.tile([128, GW], f32, tag="v")
                    for a in range(KA):
                        nc.tensor.matmul(
                            pq,
                            lhsT=xtok[:, a, :],
                            rhs=wq_sb[:, a, :],
                            start=(a == 0),
                            stop=(a == KA - 1),
                        )
                    for a in range(KA):
                        nc.tensor.matmul(
                            pv,
                            lhsT=xtok[:, a, :],
                            rhs=wv_sb[:, a, :],
                            start=(a == 0),
                            stop=(a == KA - 1),
                        )
                    nc.vector.tensor_copy(
                        out=vp_sb[:, t, :, 0:64],
                        in_=pv.rearrange("p (h d) -> p h d", h=HG),
                    )
                    norm_rope_transpose(pq, "q", cosq_sb, sinq_sb, qt_sb, t)

            # -------- Phase 2+3: attention (stripe-outer) + out proj --------
            with (
                tc.tile_pool(name="ebuf", bufs=2) as ebuf,
                tc.tile_pool(name="lrow", bufs=2) as lpool,
                tc.tile_pool(name="linv", bufs=2) as lsp,
                tc.tile_pool(name="obuf", bufs=3) as obuf,
                tc.tile_pool(name="ps_s", bufs=2, space="PSUM") as pss,
                tc.tile_pool(name="ps_o", bufs=1, space="PSUM") as pso,
                tc.tile_pool(name="ps_out", bufs=2, space="PSUM") as psout,
            ):
                for s in range(2):
                    q0 = s * 1024
                    for h in range(HG):
                        hp = (h % 2) * 64
                        hj = h // 2
                        E = ebuf.tile([128, NT, 1024], bf16, tag="E")
                        for i in range(NT):
                            pscore = pss.tile([128, 1024], f32, tag="s")
                            for c in range(2):
                                nc.tensor.matmul(
                                    pscore[:, c * 512 : c * 512 + 512],
                                    lhsT=kt_sb[
                                        hp : hp + 64, hj, i * 128 : i * 128 + 128
                                    ].bitcast(f32r),
                                    rhs=qt_sb[
                                        hp : hp + 64,
                                        hj,
                                        q0 + c * 512 : q0 + c * 512 + 512,
                                    ].bitcast(f32r),
                                    start=True,
                                    stop=True,
                                )
                            nc.scalar.activation(
                                out=E[:, i, :],
                                in_=pscore,
                                func=mybir.ActivationFunctionType.Exp,
                                scale=SCALE,
                            )
                        po = pso.tile([65, 1024], f32, tag="po")
                        for i in range(NT):
                            for c in range(2):
                                nc.tensor.matmul(
                                    po[:, c * 512 : c * 512 + 512],
                                    lhsT=vp_sb[:, i, h, 0:65],
                                    rhs=E[:, i, c * 512 : c * 512 + 512],
                                    start=(i == 0),
                                    stop=(i == NT - 1),
                                )
                        lraw = lpool.tile([1, 1024], f32, tag="lraw")
                        nc.vector.tensor_copy(out=lraw, in_=po[64:65, :])
                        lrow = lpool.tile([1, 1024], f32, tag="lrow")
                        nc.vector.reciprocal_approx_fast(out=lrow, in_=lraw)
                        li = lsp.tile([64, 1024], f32, tag="linv")
                        nc.sync.dma_start(out=li, in_=bc_h(lrow[0:1, :], 64))
                        nc.vector.tensor_tensor(
                            out=xat_sb[hp : hp + 64, hj, q0 : q0 + 1024].bitcast(
                                f32r
                            ),
                            in0=po[0:64, :],
                            in1=li,
                            op=mult,
                        )
                    # out projection for this stripe's token tiles
                    for t in range(s * 8, s * 8 + 8):
                        ot = obuf.tile([128, 1024], f32, tag="ot")
                        for c2 in range(2):
                            po2 = psout.tile([128, 512], f32, tag="o")
                            for j in range(2):
                                nc.tensor.matmul(
                                    po2,
                                    lhsT=xat_sb[
                                        :, j, t * 128 : t * 128 + 128
                                    ].bitcast(f32r),
                                    rhs=wo_sb[:, j, c2 * 512 : c2 * 512 + 512],
                                    start=(j == 0),
                                    stop=(j == 1),
                                )
                            nc.vector.tensor_copy(
                                out=ot[:, c2 * 512 : c2 * 512 + 512], in_=po2
                            )
                        nc.sync.dma_start(
                            out=out_d[t * 128 : t * 128 + 128, :], in_=ot
                        )

    nc.compile()
    return nc


def _prep_inputs(hidden_states, Wq, Wk, Wv, Wo, q_norm_w, k_norm_w, cos, sin):
    cos2 = np.ascontiguousarray(np.asarray(cos, np.float32).reshape(N, D))
    sin2 = np.ascontiguousarray(np.asarray(sin, np.float32).reshape(N, D))

    def rope_tables(w):
        cw = (cos2 * w[None, :]).reshape(NT, 128, D)
        sw = np.empty((N, D), np.float32)
        sw[:, : D // 2] = -sin2[:, : D // 2] * w[None, D // 2 :]
        sw[:, D // 2 :] = sin2[:, D // 2 :] * w[None, : D // 2]
        return np.ascontiguousarray(cw), np.ascontiguousarray(
            sw.reshape(NT, 128, D)
        )

    cq, sq = rope_tables(np.asarray(q_norm_w, np.float32))
    ck, sk = rope_tables(np.asarray(k_norm_w, np.float32))

    in_maps = []
    for core in range(8):
        b, g = core // 4, core % 4
        ht = np.ascontiguousarray(np.asarray(hidden_states[b], np.float32).T)
        xt = np.ascontiguousarray(
            ht.reshape(KA, 128, NT, 128).transpose(2, 0, 1, 3)
        )
        sl = slice(g * GW, (g + 1) * GW)
        in_maps.append(
            dict(
                xt=xt,
                wq=np.ascontiguousarray(np.asarray(Wq[:, sl], np.float32)).reshape(
                    KA, 128, GW
                ),
                wk=np.ascontiguousarray(np.asarray(Wk[:, sl], np.float32)).reshape(
                    KA, 128, GW
                ),
                wv=np.ascontiguousarray(np.asarray(Wv[:, sl], np.float32)).reshape(
                    KA, 128, GW
                ),
                wo=np.ascontiguousarray(np.asarray(Wo[sl, :], np.float32)).reshape(
                    2, 128, C
                ),
                cosq=cq,
                sinq=sq,
                cosk=ck,
                sink=sk,
            )
        )
    return in_maps


def run(trace=False, **inputs):
    from concourse.bass_utils import run_bass_kernel_spmd

    if "nc" not in _CACHE:
        _CACHE["nc"] = _build()
    nc = _CACHE["nc"]

    in_maps = _prep_inputs(
        inputs["hidden_states"],
        inputs["Wq"],
        inputs["Wk"],
        inputs["Wv"],
        inputs["Wo"],
        inputs["q_norm_w"],
        inputs["k_norm_w"],
        inputs["cos"],
        inputs["sin"],
    )
    res = run_bass_kernel_spmd(nc, in_maps, core_ids=list(range(8)), trace=trace)
    bo = np.asarray(inputs["bo"], np.float32)
    out = np.empty((B, N, C), np.float32)
    for b in range(B):
        acc = res.results[b * 4]["out"].astype(np.float32).copy()
        for g in range(1, 4):
            acc += res.results[b * 4 + g]["out"]
        out[b] = acc + bo[None, :]
    return out, res


def kernel(**inputs):
    out, _ = run(trace=False, **inputs)
    return out


# revision 13
# speedup vs baseline: 1.0303x; 1.0303x over previous
"""Trainium2 Bass kernel for nn_Attention_17995912970332.

Full attention layer: QKV proj + per-head RMSNorm(q,k) + RoPE + softmax
attention + output projection. B=2, N=2048, C=1024, H=16 heads, D=64.

Sharding: 8 cores = 2 (batch) x 4 (head groups of 4 heads). Each core
computes its 4 heads' attention and a partial output projection
(row-parallel Wo); host sums the 4 partials per batch and adds bias.

Structure (software-pipelined):
  Pass A: K projection for all 16 token tiles -> rmsnorm+rope -> PE-transpose
          into kt [d, pair, ntok].  (x^T streamed from HBM)
  Pass B: Q + V projections per token tile (x^T streamed again); qt filled
          progressively so attention stripe s=0 starts once tiles 0-7 done.
  Attention (stripe-outer): for s in {0,1}: for h in {0..3}:
      S^T = kt_i^T-contract matmul (K=64, fp32r), exp on ScalarE
      (scale=1/8 folded) -> bf16 E; PV with lhsT=[V|1] bf16 -> po rows 0..63 =
      unnormalized O^T, row 64 = softmax denom l; linv=reciprocal_approx_fast;
      broadcast linv across 64 partitions with a partition-stride-0 SBUF DMA;
      xat = po * linv.
    then output projection for the 8 token tiles of stripe s (fp32r) -> DMA.
"""

import sys

import numpy as np

sys.path.insert(0, "/opt/trn_rl_repo")

H, D, EPS = 16, 64, 1e-6
SCALE = D**-0.5
B, N, C = 2, 2048, 1024
HG = 4          # heads per core
GW = HG * D     # 256, group width
NT = N // 128   # 16 token tiles
KA = C // 128   # 8 contraction tiles for qkv proj

_CACHE = {}


def _build():
    import concourse.bass as bass
    import concourse.tile as tile
    from concourse import bacc, mybir
    from concourse.masks import make_identity

    f32 = mybir.dt.float32
    f32r = mybir.dt.float32r
    bf16 = mybir.dt.bfloat16
    mult = mybir.AluOpType.mult
    add = mybir.AluOpType.add

    nc = bacc.Bacc("TRN2", target_bir_lowering=False, debug=False)

    xt_d = nc.dram_tensor("xt", [NT, KA, 128, 128], f32r, kind="ExternalInput").ap()
    wq_d = nc.dram_tensor("wq", [KA, 128, GW], f32r, kind="ExternalInput").ap()
    wk_d = nc.dram_tensor("wk", [KA, 128, GW], f32r, kind="ExternalInput").ap()
    wv_d = nc.dram_tensor("wv", [KA, 128, GW], f32r, kind="ExternalInput").ap()
    wo_d = nc.dram_tensor("wo", [2, 128, C], f32r, kind="ExternalInput").ap()
    cosq_d = nc.dram_tensor("cosq", [NT, 128, D], f32, kind="ExternalInput").ap()
    sinq_d = nc.dram_tensor("sinq", [NT, 128, D], f32, kind="ExternalInput").ap()
    cosk_d = nc.dram_tensor("cosk", [NT, 128, D], f32, kind="ExternalInput").ap()
    sink_d = nc.dram_tensor("sink", [NT, 128, D], f32, kind="ExternalInput").ap()
    out_d = nc.dram_tensor("out", [N, C], f32, kind="ExternalOutput").ap()

    def bc_h(ap2d, reps=HG):
        # [128, X] AP -> [128, reps, X] with stride-0 middle dim (broadcast)
        return bass.AP(
            tensor=ap2d.tensor,
            offset=ap2d.offset,
            ap=[list(ap2d.ap[0]), [0, reps], list(ap2d.ap[1])],
        )

    with tile.TileContext(nc) as tc:
        with (
            tc.tile_pool(name="consts", bufs=1) as consts,
            tc.tile_pool(name="weights", bufs=1) as wpool,
            tc.tile_pool(name="qkt", bufs=1) as qkt,
            tc.tile_pool(name="vbuf", bufs=1) as vbuf,
            tc.tile_pool(name="xat", bufs=1) as xatp,
            tc.tile_pool(name="cs4", bufs=1) as cs4,
        ):
            ident = consts.tile([128, 128], f32)
            make_identity(nc, ident)
            eps128 = consts.tile([128, 1], f32)
            nc.vector.memset(eps128, EPS)

            wq_sb = wpool.tile([128, KA, GW], f32r, tag="wq")
            wk_sb = wpool.tile([128, KA, GW], f32r, tag="wk")
            wv_sb = wpool.tile([128, KA, GW], f32r, tag="wv")
            wo_sb = wpool.tile([128, 2, C], f32r, tag="wo")
            nc.sync.dma_start(out=wk_sb, in_=wk_d.rearrange("a p n -> p a n"))
            nc.sync.dma_start(out=wq_sb, in_=wq_d.rearrange("a p n -> p a n"))
            nc.sync.dma_start(out=wv_sb, in_=wv_d.rearrange("a p n -> p a n"))
            nc.sync.dma_start(out=wo_sb, in_=wo_d.rearrange("a p n -> p a n"))

            cosq_sb = cs4.tile([128, NT, D], f32, tag="cq")
            sinq_sb = cs4.tile([128, NT, D], f32, tag="sq")
            cosk_sb = cs4.tile([128, NT, D], f32, tag="ck")
            sink_sb = cs4.tile([128, NT, D], f32, tag="sk")
            nc.sync.dma_start(out=cosk_sb, in_=cosk_d.rearrange("t p d -> p t d"))
            nc.sync.dma_start(out=sink_sb, in_=sink_d.rearrange("t p d -> p t d"))
            nc.sync.dma_start(out=cosq_sb, in_=cosq_d.rearrange("t p d -> p t d"))
            nc.sync.dma_start(out=sinq_sb, in_=sinq_d.rearrange("t p d -> p t d"))

            qt_s0 = qkt.tile([128, 2, 1024], f32, tag="qt0")
            qt_s1 = qkt.tile([128, 2, 1024], f32, tag="qt1")
            qt_s = [qt_s0, qt_s1]  # q^T: p=(h%2)*64+d, j=h//2
            kt_sb = qkt.tile([128, 2, N], f32, tag="kt")
            xat_s0 = xatp.tile([128, 2, 1024], f32, tag="xat0")
            xat_s1 = xatp.tile([128, 2, 1024], f32, tag="xat1")
            xat_s = [xat_s0, xat_s1]
            vp_sb = vbuf.tile([128, NT, HG, 66], bf16, tag="vp")
            nc.vector.memset(vp_sb, 1.0)

            # -------- Phase 1: two passes over x (K first, then Q+V) --------
            with (
                tc.tile_pool(name="xtok", bufs=3) as xpool,
                tc.tile_pool(name="p1w", bufs=3) as p1w,
                tc.tile_pool(name="p1s", bufs=4) as p1s,
                tc.tile_pool(name="ps_qkv", bufs=2, space="PSUM") as pqkv,
                tc.tile_pool(name="ps_tr", bufs=2, space="PSUM") as ptr,
            ):

                def norm_rope_transpose(p, nm, cos_sb, sin_sb, dst, t, col):
                    p3 = p.rearrange("p (h d) -> p h d", h=HG)
                    sq = p1s.tile([128, GW], f32, tag="sq")
                    nc.scalar.activation(
                        out=sq, in_=p, func=mybir.ActivationFunctionType.Square
                    )
                    ms = p1s.tile([128, HG], f32, tag="ms")
                    nc.vector.tensor_reduce(
                        out=ms,
                        in_=sq.rearrange("p (h d) -> p h d", h=HG),
                        axis=mybir.AxisListType.X,
                        op=add,
                    )
                    rstd = p1s.tile([128, HG], f32, tag="rstd")
                    nc.scalar.activation(
                        out=rstd,
                        in_=ms,
                        func=mybir.ActivationFunctionType.Sqrt,
                        scale=1.0 / D,
                        bias=eps128,
                    )
                    nc.vector.reciprocal(out=rstd, in_=rstd)
                    rotm = p1w.tile([128, HG, D], f32, tag="rotm" + nm)
                    nc.vector.tensor_tensor(
                        out=rotm[:, :, 0:32],
                        in0=p3[:, :, 32:64],
                        in1=bc_h(sin_sb[:, t, 0:32]),
                        op=mult,
                    )
                    nc.vector.tensor_tensor(
                        out=rotm[:, :, 32:64],
                        in0=p3[:, :, 0:32],
                        in1=bc_h(sin_sb[:, t, 32:64]),
                        op=mult,
                    )
                    qc = p1w.tile([128, HG, D], f32, tag="qc" + nm)
                    nc.vector.tensor_tensor(
                        out=qc, in0=p3, in1=bc_h(cos_sb[:, t, :]), op=mult
                    )
                    qf = p1w.tile([128, HG, D], f32, tag="qf" + nm)
                    nc.gpsimd.tensor_tensor(out=qf, in0=qc, in1=rotm, op=add)
                    for h in range(HG):
                        nc.vector.tensor_scalar_mul(
                            qf[:, h, :], qf[:, h, :], rstd[:, h : h + 1]
                        )
                    qf2 = qf.rearrange("p h d -> p (h d)")
                    for half in range(2):
                        ptile = ptr.tile([128, 128], f32, tag="tr")
                        nc.tensor.transpose(
                            out=ptile,
                            in_=qf2[:, half * 128 : half * 128 + 128],
                            identity=ident,
                        )
                        nc.vector.tensor_copy(
                            out=dst[:, half, col : col + 128].bitcast(f32r),
                            in_=ptile,
                        )

                # Pass A: K for all tiles
                for t in range(NT):
                    xtok = xpool.tile([128, KA, 128], f32r, tag="xtok")
                    nc.sync.dma_start(
                        out=xtok, in_=xt_d[t].rearrange("a p n -> p a n")
                    )
                    p = pqkv.tile([128, GW], f32, tag="k")
                    for a in range(KA):
                        nc.tensor.matmul(
                            p,
                            lhsT=xtok[:, a, :],
                            rhs=wk_sb[:, a, :],
                            start=(a == 0),
                            stop=(a == KA - 1),
                        )
                    norm_rope_transpose(p, "k", cosk_sb, sink_sb, kt_sb, t, t * 128)

                # Pass B: Q + V per tile
                for t in range(NT):
                    xtok = xpool.tile([128, KA, 128], f32r, tag="xtok")
                    nc.sync.dma_start(
                        out=xtok, in_=xt_d[t].rearrange("a p n -> p a n")
                    )
                    pq = pqkv.tile([128, GW], f32, tag="q")
                    pv = pqkv.tile([128, GW], f32, tag="v")
                    for a in range(KA):
                        nc.tensor.matmul(
                            pq,
                            lhsT=xtok[:, a, :],
                            rhs=wq_sb[:, a, :],
                            start=(a == 0),
                            stop=(a == KA - 1),
                        )
                    for a in range(KA):
                        nc.tensor.matmul(
                            pv,
                            lhsT=xtok[:, a, :],
                            rhs=wv_sb[:, a, :],
                            start=(a == 0),
                            stop=(a == KA - 1),
                        )
                    nc.vector.tensor_copy(
                        out=vp_sb[:, t, :, 0:64],
                        in_=pv.rearrange("p (h d) -> p h d", h=HG),
                    )
                    norm_rope_transpose(pq, "q", cosq_sb, sinq_sb, qt_s[t // 8], t, (t % 8) * 128)

            # -------- Phase 2+3: attention (stripe-outer) + out proj --------
            with (
                tc.tile_pool(name="ebuf", bufs=2) as ebuf,
                tc.tile_pool(name="lrow", bufs=2) as lpool,
                tc.tile_pool(name="linv", bufs=2) as lsp,
                tc.tile_pool(name="obuf", bufs=3) as obuf,
                tc.tile_pool(name="ps_s", bufs=2, space="PSUM") as pss,
                tc.tile_pool(name="ps_o", bufs=1, space="PSUM") as pso,
                tc.tile_pool(name="ps_out", bufs=2, space="PSUM") as psout,
            ):
                for s in range(2):
                    q0 = s * 1024
                    for h in range(HG):
                        hp = (h % 2) * 64
                        hj = h // 2
                        E = ebuf.tile([128, NT, 1024], bf16, tag="E")
                        for i in range(NT):
                            pscore = pss.tile([128, 1024], f32, tag="s")
                            for c in range(2):
                                nc.tensor.matmul(
                                    pscore[:, c * 512 : c * 512 + 512],
                                    lhsT=kt_sb[
                                        hp : hp + 64, hj, i * 128 : i * 128 + 128
                                    ].bitcast(f32r),
                                    rhs=qt_s[s][
                                        hp : hp + 64, hj, c * 512 : c * 512 + 512
                                    ].bitcast(f32r),
                                    start=True,
                                    stop=True,
                                )
                            nc.scalar.activation(
                                out=E[:, i, :],
                                in_=pscore,
                                func=mybir.ActivationFunctionType.Exp,
                                scale=SCALE,
                            )
                        po = pso.tile([65, 1024], f32, tag="po")
                        for i in range(NT):
                            for c in range(2):
                                nc.tensor.matmul(
                                    po[:, c * 512 : c * 512 + 512],
                                    lhsT=vp_sb[:, i, h, 0:65],
                                    rhs=E[:, i, c * 512 : c * 512 + 512],
                                    start=(i == 0),
                                    stop=(i == NT - 1),
                                )
                        lraw = lpool.tile([1, 1024], f32, tag="lraw")
                        nc.vector.tensor_copy(out=lraw, in_=po[64:65, :])
                        lrow = lpool.tile([1, 1024], f32, tag="lrow")
                        nc.vector.reciprocal_approx_fast(out=lrow, in_=lraw)
                        li = lsp.tile([64, 1024], f32, tag="linv")
                        nc.sync.dma_start(out=li, in_=bc_h(lrow[0:1, :], 64))
                        nc.vector.tensor_tensor(
                            out=xat_s[s][hp : hp + 64, hj, :].bitcast(f32r),
                            in0=po[0:64, :],
                            in1=li,
                            op=mult,
                        )
                    # out projection for this stripe's token tiles
                    for t in range(s * 8, s * 8 + 8):
                        ot = obuf.tile([128, 1024], f32, tag="ot")
                        for c2 in range(2):
                            po2 = psout.tile([128, 512], f32, tag="o")
                            for j in range(2):
                                nc.tensor.matmul(
                                    po2,
                                    lhsT=xat_s[s][
                                        :, j, (t % 8) * 128 : (t % 8) * 128 + 128
                                    ].bitcast(f32r),
                                    rhs=wo_sb[:, j, c2 * 512 : c2 * 512 + 512],
                                    start=(j == 0),
                                    stop=(j == 1),
                                )
                            nc.vector.tensor_copy(
                                out=ot[:, c2 * 512 : c2 * 512 + 512], in_=po2
                            )
                        nc.sync.dma_start(
                            out=out_d[t * 128 : t * 128 + 128, :], in_=ot
                        )

    nc.compile()
    return nc


def _prep_inputs(hidden_states, Wq, Wk, Wv, Wo, q_norm_w, k_norm_w, cos, sin):
    cos2 = np.ascontiguousarray(np.asarray(cos, np.float32).reshape(N, D))
    sin2 = np.ascontiguousarray(np.asarray(sin, np.float32).reshape(N, D))

    def rope_tables(w):
        cw = (cos2 * w[None, :]).reshape(NT, 128, D)
        sw = np.empty((N, D), np.float32)
        sw[:, : D // 2] = -sin2[:, : D // 2] * w[None, D // 2 :]
        sw[:, D // 2 :] = sin2[:, D // 2 :] * w[None, : D // 2]
        return np.ascontiguousarray(cw), np.ascontiguousarray(
            sw.reshape(NT, 128, D)
        )

    cq, sq = rope_tables(np.asarray(q_norm_w, np.float32))
    ck, sk = rope_tables(np.asarray(k_norm_w, np.float32))

    in_maps = []
    for core in range(8):
        b, g = core // 4, core % 4
        ht = np.ascontiguousarray(np.asarray(hidden_states[b], np.float32).T)
        xt = np.ascontiguousarray(
            ht.reshape(KA, 128, NT, 128).transpose(2, 0, 1, 3)
        )
        sl = slice(g * GW, (g + 1) * GW)
        in_maps.append(
            dict(
                xt=xt,
                wq=np.ascontiguousarray(np.asarray(Wq[:, sl], np.float32)).reshape(
                    KA, 128, GW
                ),
                wk=np.ascontiguousarray(np.asarray(Wk[:, sl], np.float32)).reshape(
                    KA, 128, GW
                ),
                wv=np.ascontiguousarray(np.asarray(Wv[:, sl], np.float32)).reshape(
                    KA, 128, GW
                ),
                wo=np.ascontiguousarray(np.asarray(Wo[sl, :], np.float32)).reshape(
                    2, 128, C
                ),
                cosq=cq,
                sinq=sq,
                cosk=ck,
                sink=sk,
            )
        )
    return in_maps


def run(trace=False, **inputs):
    from concourse.bass_utils import run_bass_kernel_spmd

    if "nc" not in _CACHE:
        _CACHE["nc"] = _build()
    nc = _CACHE["nc"]

    in_maps = _prep_inputs(
        inputs["hidden_states"],
        inputs["Wq"],
        inputs["Wk"],
        inputs["Wv"],
        inputs["Wo"],
        inputs["q_norm_w"],
        inputs["k_norm_w"],
        inputs["cos"],
        inputs["sin"],
    )
    res = run_bass_kernel_spmd(nc, in_maps, core_ids=list(range(8)), trace=trace)
    bo = np.asarray(inputs["bo"], np.float32)
    out = np.empty((B, N, C), np.float32)
    for b in range(B):
        acc = res.results[b * 4]["out"].astype(np.float32).copy()
        for g in range(1, 4):
            acc += res.results[b * 4 + g]["out"]
        out[b] = acc + bo[None, :]
    return out, res


def kernel(**inputs):
    out, _ = run(trace=False, **inputs)
    return out


# revision 14
# speedup vs baseline: 1.0938x; 1.0616x over previous
"""Trainium2 Bass kernel for nn_Attention_17995912970332.

Full attention layer: QKV proj + per-head RMSNorm(q,k) + RoPE + softmax
attention + output projection. B=2, N=2048, C=1024, H=16 heads, D=64.

Sharding: 8 cores = 2 (batch) x 4 (head groups of 4 heads). Each core
computes its 4 heads' attention and a partial output projection
(row-parallel Wo); host sums the 4 partials per batch and adds bias.

Structure (software-pipelined):
  Pass A: K projection for all 16 token tiles -> rmsnorm+rope -> PE-transpose
          into kt [d, pair, ntok].  (x^T streamed from HBM)
  Pass B: Q + V projections per token tile (x^T streamed again); qt filled
          progressively so attention stripe s=0 starts once tiles 0-7 done.
  Attention (stripe-outer): for s in {0,1}: for h in {0..3}:
      S^T = kt_i^T-contract matmul (K=64, fp32r), exp on ScalarE
      (scale=1/8 folded) -> bf16 E; PV with lhsT=[V|1] bf16 -> po rows 0..63 =
      unnormalized O^T, row 64 = softmax denom l; linv=reciprocal_approx_fast;
      broadcast linv across 64 partitions with a partition-stride-0 SBUF DMA;
      xat = po * linv.
    then output projection for the 8 token tiles of stripe s (fp32r) -> DMA.
"""

import sys

import numpy as np

sys.path.insert(0, "/opt/trn_rl_repo")

H, D, EPS = 16, 64, 1e-6
SCALE = D**-0.5
B, N, C = 2, 2048, 1024
HG = 4          # heads per core
GW = HG * D     # 256, group width
NT = N // 128   # 16 token tiles
KA = C // 128   # 8 contraction tiles for qkv proj

_CACHE = {}


def _build():
    import concourse.bass as bass
    import concourse.tile as tile
    from concourse import bacc, mybir
    from concourse.masks import make_identity

    f32 = mybir.dt.float32
    f32r = mybir.dt.float32r
    bf16 = mybir.dt.bfloat16
    mult = mybir.AluOpType.mult
    add = mybir.AluOpType.add

    nc = bacc.Bacc("TRN2", target_bir_lowering=False, debug=False)

    xt_d = nc.dram_tensor("xt", [NT, KA, 128, 128], f32r, kind="ExternalInput").ap()
    wq_d = nc.dram_tensor("wq", [KA, 128, GW], f32r, kind="ExternalInput").ap()
    wk_d = nc.dram_tensor("wk", [KA, 128, GW], f32r, kind="ExternalInput").ap()
    wv_d = nc.dram_tensor("wv", [KA, 128, GW], f32r, kind="ExternalInput").ap()
    wo_d = nc.dram_tensor("wo", [2, 128, C], f32r, kind="ExternalInput").ap()
    cosq_d = nc.dram_tensor("cosq", [NT, 128, D], f32, kind="ExternalInput").ap()
    sinq_d = nc.dram_tensor("sinq", [NT, 128, D], f32, kind="ExternalInput").ap()
    cosk_d = nc.dram_tensor("cosk", [NT, 128, D], f32, kind="ExternalInput").ap()
    sink_d = nc.dram_tensor("sink", [NT, 128, D], f32, kind="ExternalInput").ap()
    out_d = nc.dram_tensor("out", [N, C], f32, kind="ExternalOutput").ap()

    def bc_h(ap2d, reps=HG):
        # [128, X] AP -> [128, reps, X] with stride-0 middle dim (broadcast)
        return bass.AP(
            tensor=ap2d.tensor,
            offset=ap2d.offset,
            ap=[list(ap2d.ap[0]), [0, reps], list(ap2d.ap[1])],
        )

    with tile.TileContext(nc) as tc:
        with (
            tc.tile_pool(name="consts", bufs=1) as consts,
            tc.tile_pool(name="weights", bufs=1) as wpool,
            tc.tile_pool(name="qkt", bufs=1) as qkt,
            tc.tile_pool(name="vbuf", bufs=1) as vbuf,
            tc.tile_pool(name="xat", bufs=1) as xatp,
            tc.tile_pool(name="cs4", bufs=1) as cs4,
        ):
            ident = consts.tile([128, 128], f32)
            make_identity(nc, ident)
            eps128 = consts.tile([128, 1], f32)
            nc.vector.memset(eps128, EPS)

            wq_sb = wpool.tile([128, KA, GW], f32r, tag="wq")
            wk_sb = wpool.tile([128, KA, GW], f32r, tag="wk")
            wv_sb = wpool.tile([128, KA, GW], f32r, tag="wv")
            wo_sb = wpool.tile([128, 2, C], f32r, tag="wo")
            nc.sync.dma_start(out=wk_sb, in_=wk_d.rearrange("a p n -> p a n"))
            nc.sync.dma_start(out=wq_sb, in_=wq_d.rearrange("a p n -> p a n"))
            nc.sync.dma_start(out=wv_sb, in_=wv_d.rearrange("a p n -> p a n"))
            nc.sync.dma_start(out=wo_sb, in_=wo_d.rearrange("a p n -> p a n"))

            cosq_sb = cs4.tile([128, NT, D], f32, tag="cq")
            sinq_sb = cs4.tile([128, NT, D], f32, tag="sq")
            cosk_sb = cs4.tile([128, NT, D], f32, tag="ck")
            sink_sb = cs4.tile([128, NT, D], f32, tag="sk")
            nc.sync.dma_start(out=cosk_sb, in_=cosk_d.rearrange("t p d -> p t d"))
            nc.sync.dma_start(out=sink_sb, in_=sink_d.rearrange("t p d -> p t d"))
            nc.sync.dma_start(out=cosq_sb, in_=cosq_d.rearrange("t p d -> p t d"))
            nc.sync.dma_start(out=sinq_sb, in_=sinq_d.rearrange("t p d -> p t d"))

            qt_s0 = qkt.tile([128, 2, 1024], f32, tag="qt0")
            qt_s1 = qkt.tile([128, 2, 1024], f32, tag="qt1")
            qt_s = [qt_s0, qt_s1]  # q^T: p=(h%2)*64+d, j=h//2
            kt_sb = qkt.tile([128, 2, N], f32, tag="kt")
            xat_s0 = xatp.tile([128, 2, 1024], f32, tag="xat0")
            xat_s1 = xatp.tile([128, 2, 1024], f32, tag="xat1")
            xat_s = [xat_s0, xat_s1]
            vp_sb = vbuf.tile([128, NT, HG, 66], bf16, tag="vp")
            nc.vector.memset(vp_sb, 1.0)

            # -------- Phase 1: two passes over x (K first, then Q+V) --------
            with (
                tc.tile_pool(name="xtok", bufs=3) as xpool,
                tc.tile_pool(name="p1w", bufs=3) as p1w,
                tc.tile_pool(name="p1s", bufs=4) as p1s,
                tc.tile_pool(name="ps_qkv", bufs=1, space="PSUM") as pqkv,
                tc.tile_pool(name="ps_tr", bufs=1, space="PSUM") as ptr,
            ):

                def norm_rope_transpose(p, nm, cos_sb, sin_sb, dst, t, col):
                    p3 = p.rearrange("p (h d) -> p h d", h=HG)
                    sq = p1s.tile([128, GW], f32, tag="sq")
                    nc.scalar.activation(
                        out=sq, in_=p, func=mybir.ActivationFunctionType.Square
                    )
                    ms = p1s.tile([128, HG], f32, tag="ms")
                    nc.vector.tensor_reduce(
                        out=ms,
                        in_=sq.rearrange("p (h d) -> p h d", h=HG),
                        axis=mybir.AxisListType.X,
                        op=add,
                    )
                    rstd = p1s.tile([128, HG], f32, tag="rstd")
                    nc.scalar.activation(
                        out=rstd,
                        in_=ms,
                        func=mybir.ActivationFunctionType.Sqrt,
                        scale=1.0 / D,
                        bias=eps128,
                    )
                    nc.vector.reciprocal(out=rstd, in_=rstd)
                    rotm = p1w.tile([128, HG, D], f32, tag="rotm" + nm)
                    nc.vector.tensor_tensor(
                        out=rotm[:, :, 0:32],
                        in0=p3[:, :, 32:64],
                        in1=bc_h(sin_sb[:, t, 0:32]),
                        op=mult,
                    )
                    nc.vector.tensor_tensor(
                        out=rotm[:, :, 32:64],
                        in0=p3[:, :, 0:32],
                        in1=bc_h(sin_sb[:, t, 32:64]),
                        op=mult,
                    )
                    qc = p1w.tile([128, HG, D], f32, tag="qc" + nm)
                    nc.vector.tensor_tensor(
                        out=qc, in0=p3, in1=bc_h(cos_sb[:, t, :]), op=mult
                    )
                    qf = p1w.tile([128, HG, D], f32, tag="qf" + nm)
                    nc.gpsimd.tensor_tensor(out=qf, in0=qc, in1=rotm, op=add)
                    for h in range(HG):
                        nc.vector.tensor_scalar_mul(
                            qf[:, h, :], qf[:, h, :], rstd[:, h : h + 1]
                        )
                    qf2 = qf.rearrange("p h d -> p (h d)")
                    for half in range(2):
                        ptile = ptr.tile([128, 128], f32, tag="tr")
                        nc.tensor.transpose(
                            out=ptile,
                            in_=qf2[:, half * 128 : half * 128 + 128],
                            identity=ident,
                        )
                        nc.vector.tensor_copy(
                            out=dst[:, half, col : col + 128].bitcast(f32r),
                            in_=ptile,
                        )

                # Pass A: K for all tiles
                for t in range(NT):
                    xtok = xpool.tile([128, KA, 128], f32r, tag="xtok")
                    nc.sync.dma_start(
                        out=xtok, in_=xt_d[t].rearrange("a p n -> p a n")
                    )
                    p = pqkv.tile([128, GW], f32, tag="k")
                    for a in range(KA):
                        nc.tensor.matmul(
                            p,
                            lhsT=xtok[:, a, :],
                            rhs=wk_sb[:, a, :],
                            start=(a == 0),
                            stop=(a == KA - 1),
                        )
                    norm_rope_transpose(p, "k", cosk_sb, sink_sb, kt_sb, t, t * 128)

                # Pass B: Q + V per tile
                for t in range(NT):
                    xtok = xpool.tile([128, KA, 128], f32r, tag="xtok")
                    nc.sync.dma_start(
                        out=xtok, in_=xt_d[t].rearrange("a p n -> p a n")
                    )
                    pq = pqkv.tile([128, GW], f32, tag="q")
                    pv = pqkv.tile([128, GW], f32, tag="v")
                    for a in range(KA):
                        nc.tensor.matmul(
                            pq,
                            lhsT=xtok[:, a, :],
                            rhs=wq_sb[:, a, :],
                            start=(a == 0),
                            stop=(a == KA - 1),
                        )
                    for a in range(KA):
                        nc.tensor.matmul(
                            pv,
                            lhsT=xtok[:, a, :],
                            rhs=wv_sb[:, a, :],
                            start=(a == 0),
                            stop=(a == KA - 1),
                        )
                    nc.vector.tensor_copy(
                        out=vp_sb[:, t, :, 0:64],
                        in_=pv.rearrange("p (h d) -> p h d", h=HG),
                    )
                    norm_rope_transpose(pq, "q", cosq_sb, sinq_sb, qt_s[t // 8], t, (t % 8) * 128)

            # -------- Phase 2+3: attention (stripe-outer) + out proj --------
            with (
                tc.tile_pool(name="ebuf", bufs=2) as ebuf,
                tc.tile_pool(name="lrow", bufs=2) as lpool,
                tc.tile_pool(name="linv", bufs=2) as lsp,
                tc.tile_pool(name="obuf", bufs=3) as obuf,
                tc.tile_pool(name="ps_s", bufs=2, space="PSUM") as pss,
                tc.tile_pool(name="ps_o", bufs=1, space="PSUM") as pso,
                tc.tile_pool(name="ps_out", bufs=2, space="PSUM") as psout,
            ):
                for s in range(2):
                    q0 = s * 1024
                    for h in range(HG):
                        hp = (h % 2) * 64
                        hj = h // 2
                        E = ebuf.tile([128, NT, 1024], bf16, tag="E")
                        for i in range(NT):
                            pscore = pss.tile([128, 1024], f32, tag="s")
                            for c in range(2):
                                nc.tensor.matmul(
                                    pscore[:, c * 512 : c * 512 + 512],
                                    lhsT=kt_sb[
                                        hp : hp + 64, hj, i * 128 : i * 128 + 128
                                    ].bitcast(f32r),
                                    rhs=qt_s[s][
                                        hp : hp + 64, hj, c * 512 : c * 512 + 512
                                    ].bitcast(f32r),
                                    start=True,
                                    stop=True,
                                )
                            nc.scalar.activation(
                                out=E[:, i, :],
                                in_=pscore,
                                func=mybir.ActivationFunctionType.Exp,
                                scale=SCALE,
                            )
                        po = pso.tile([65, 1024], f32, tag="po")
                        for i in range(NT):
                            for c in range(2):
                                nc.tensor.matmul(
                                    po[:, c * 512 : c * 512 + 512],
                                    lhsT=vp_sb[:, i, h, 0:65],
                                    rhs=E[:, i, c * 512 : c * 512 + 512],
                                    start=(i == 0),
                                    stop=(i == NT - 1),
                                )
                        lraw = lpool.tile([1, 1024], f32, tag="lraw")
                        nc.vector.tensor_copy(out=lraw, in_=po[64:65, :])
                        lrow = lpool.tile([1, 1024], f32, tag="lrow")
                        nc.vector.reciprocal_approx_fast(out=lrow, in_=lraw)
                        li = lsp.tile([64, 1024], f32, tag="linv")
                        nc.sync.dma_start(out=li, in_=bc_h(lrow[0:1, :], 64))
                        nc.vector.tensor_tensor(
                            out=xat_s[s][hp : hp + 64, hj, :].bitcast(f32r),
                            in0=po[0:64, :],
                            in1=li,
                            op=mult,
                        )
                    # out projection for this stripe's token tiles
                    for t in range(s * 8, s * 8 + 8):
                        ot = obuf.tile([128, 1024], f32, tag="ot")
                        for c2 in range(2):
                            po2 = psout.tile([128, 512], f32, tag="o")
                            for j in range(2):
                                nc.tensor.matmul(
                                    po2,
                                    lhsT=xat_s[s][
                                        :, j, (t % 8) * 128 : (t % 8) * 128 + 128
                                    ].bitcast(f32r),
                                    rhs=wo_sb[:, j, c2 * 512 : c2 * 512 + 512],
                                    start=(j == 0),
                                    stop=(j == 1),
                                )
                            if (t + c2) % 2 == 0:
                                nc.vector.tensor_copy(
                                    out=ot[:, c2 * 512 : c2 * 512 + 512], in_=po2
                                )
                            else:
                                nc.scalar.copy(
                                    out=ot[:, c2 * 512 : c2 * 512 + 512], in_=po2
                                )
                        nc.sync.dma_start(
                            out=out_d[t * 128 : t * 128 + 128, :], in_=ot
                        )

    nc.compile()
    return nc


def _prep_inputs(hidden_states, Wq, Wk, Wv, Wo, q_norm_w, k_norm_w, cos, sin):
    cos2 = np.ascontiguousarray(np.asarray(cos, np.float32).reshape(N, D))
    sin2 = np.ascontiguousarray(np.asarray(sin, np.float32).reshape(N, D))

    def rope_tables(w):
        cw = (cos2 * w[None, :]).reshape(NT, 128, D)
        sw = np.empty((N, D), np.float32)
        sw[:, : D // 2] = -sin2[:, : D // 2] * w[None, D // 2 :]
        sw[:, D // 2 :] = sin2[:, D // 2 :] * w[None, : D // 2]
        return np.ascontiguousarray(cw), np.ascontiguousarray(
            sw.reshape(NT, 128, D)
        )

    cq, sq = rope_tables(np.asarray(q_norm_w, np.float32))
    ck, sk = rope_tables(np.asarray(k_norm_w, np.float32))

    in_maps = []
    for core in range(8):
        b, g = core // 4, core % 4
        ht = np.ascontiguousarray(np.asarray(hidden_states[b], np.float32).T)
        xt = np.ascontiguousarray(
            ht.reshape(KA, 128, NT, 128).transpose(2, 0, 1, 3)
        )
        sl = slice(g * GW, (g + 1) * GW)
        in_maps.append(
            dict(
                xt=xt,
                wq=np.ascontiguousarray(np.asarray(Wq[:, sl], np.float32)).reshape(
                    KA, 128, GW
                ),
                wk=np.ascontiguousarray(np.asarray(Wk[:, sl], np.float32)).reshape(
                    KA, 128, GW
                ),
                wv=np.ascontiguousarray(np.asarray(Wv[:, sl], np.float32)).reshape(
                    KA, 128, GW
                ),
                wo=np.ascontiguousarray(np.asarray(Wo[sl, :], np.float32)).reshape(
                    2, 128, C
                ),
                cosq=cq,
                sinq=sq,
                cosk=ck,
                sink=sk,
            )
        )
    return in_maps


def run(trace=False, **inputs):
    from concourse.bass_utils import run_bass_kernel_spmd

    if "nc" not in _CACHE:
        _CACHE["nc"] = _build()
    nc = _CACHE["nc"]

    in_maps = _prep_inputs(
        inputs["hidden_states"],
        inputs["Wq"],
        inputs["Wk"],
        inputs["Wv"],
        inputs["Wo"],
        inputs["q_norm_w"],
        inputs["k_norm_w"],
        inputs["cos"],
        inputs["sin"],
    )
    res = run_bass_kernel_spmd(nc, in_maps, core_ids=list(range(8)), trace=trace)
    bo = np.asarray(inputs["bo"], np.float32)
    out = np.empty((B, N, C), np.float32)
    for b in range(B):
        acc = res.results[b * 4]["out"].astype(np.float32).copy()
        for g in range(1, 4):
            acc += res.results[b * 4 + g]["out"]
        out[b] = acc + bo[None, :]
    return out, res


def kernel(**inputs):
    out, _ = run(trace=False, **inputs)
    return out
